# revision 43
# baseline (speedup 1.0000x reference)
"""Causal self-attention (B=2, T=2048, C=1024, H=16, D=64) on 8 trn2 cores.

Sharding: tensor-parallel over (batch, head-group). Core c handles batch
c//4 and heads 4*(c%4) .. 4*(c%4)+4: its 4 heads' QKV projection, causal
attention, and the partial output projection (W_proj row-shard). The 4
partials per batch are summed on the host (the Megatron all-reduce at
gather time), which also adds b_proj.

v4 design notes (what matters on this part):
  - The PE clock demotes to half speed on any idle gap and takes ~3us of
    gapless work to recover, so the PE instruction stream is explicitly
    software-pipelined: S-score matmuls for a PAIR of heads are emitted
    chunk-interleaved (row groups 0/32 run concurrently in the array),
    P@V trails S by one group, and QKV/proj fill work is quota-spread
    into the stream, weighted toward the later (ACT-bound) rounds.
  - Score path in fp8 (e4m3) with DoubleRow: x^T and wq/wk are
    pre-quantized on the host (weights scaled by 32 to sit in e4m3
    normal range; the 32*32 folds into the exp scale 1/8192). Q^T/K^T
    live as [64, 2, T] f8 tiles (heads 2m,2m+1 at partition bases 0/32,
    j=d//32) so S^T runs fp8 DoubleRow with 2x32 contraction.
  - Value path in f16 end-to-end (x^T f16, wv f16, V f16, P=exp in f16,
    P@V f16, proj f16): fp8 there costs ~2-3% output error (peaked
    softmax does not average it away), over the 2e-2 gate.
  - V' carries 64 duplicated ones-columns so P@V' accumulates the
    softmax denominator in PSUM rows 64:127; normalize is
    copy/recip/multiply on DVE (reciprocal_approx_fast misreads PSUM,
    so the denominator is copied to SBUF first).
  - Diagonal-block causal masks multiply exp output on the Pool engine
    (2 blocks batched per op); DVE carries only the PSUM-touching work.
  - y^T partials leave as f32 straight from PSUM via DMA (no engine
    downcast); host sums 4 partials per batch and adds b_proj.
"""
import os
import sys
import numpy as np

B, T, C = 2, 2048, 1024
H, D = 16, 64
HPC = 4                 # heads per core
QC = HPC * D            # 256 qkv cols per core
NCORES = 8
NT = T // 128           # 16 k-chunks of 128
NT4 = T // 512          # 4 q-chunks of 512
SCALE = 1.0 / np.sqrt(D)
WS = 32.0               # fp8 pre-scale on wq/wk + biases
EXP_SCALE = SCALE / (WS * WS)   # = 1/8192

_cache = {}


def _ensure_env():
    for p in ("/opt/trn_rl_repo", "/root/.axon_site/_ro/trn_rl_repo"):
        if os.path.isdir(p) and p not in sys.path:
            sys.path.append(p)
    jp = os.environ.get("JAX_PLATFORMS")
    if jp and "axon" not in jp and "jax" not in sys.modules:
        os.environ["JAX_PLATFORMS"] = ""


def _build():
    import concourse.bass as bass
    import concourse.bacc as bacc
    import concourse.mybir as mybir
    import concourse.tile as tile

    F32 = mybir.dt.float32
    F16 = mybir.dt.float16
    F8 = mybir.dt.float8e4
    AF = mybir.ActivationFunctionType
    DR = mybir.MatmulPerfMode.DoubleRow
    MUL = bass.mybir.AluOpType.mult
    ADD = bass.mybir.AluOpType.add

    nc = bacc.Bacc()
    xq_d = nc.dram_tensor("xq", [128, 8 * T], F8, kind="ExternalInput")
    xv_d = nc.dram_tensor("xv", [128, 8 * T], F16, kind="ExternalInput")
    wq_d = nc.dram_tensor("wq", [128, 8 * QC], F8, kind="ExternalInput")
    wk_d = nc.dram_tensor("wk", [128, 8 * QC], F8, kind="ExternalInput")
    wv_d = nc.dram_tensor("wv", [128, 8 * QC], F16, kind="ExternalInput")
    bq_d = nc.dram_tensor("bq", [128, 2], F32, kind="ExternalInput")
    bk_d = nc.dram_tensor("bk", [128, 2], F32, kind="ExternalInput")
    bv_d = nc.dram_tensor("bv", [128, QC], F32, kind="ExternalInput")
    wp_d = nc.dram_tensor("wp", [128, 2 * C], F16, kind="ExternalInput")
    mask_d = nc.dram_tensor("mask", [128, 2 * 128], F16, kind="ExternalInput")
    yt_d = nc.dram_tensor("yt", [C, T], F16, kind="ExternalOutput")

    with tile.TileContext(nc) as tc:
        with tc.tile_pool(name="cst", bufs=1) as cst, \
             tc.tile_pool(name="wgt", bufs=1) as wgt, \
             tc.tile_pool(name="qk", bufs=1) as qkp, \
             tc.tile_pool(name="vv", bufs=1) as vvp, \
             tc.tile_pool(name="pp", bufs=8) as ppp, \
             tc.tile_pool(name="dn", bufs=3) as dnp, \
             tc.tile_pool(name="yy", bufs=4) as yyp, \
             tc.tile_pool(name="mm", bufs=2, space="PSUM") as mmp, \
             tc.tile_pool(name="ss", bufs=2, space="PSUM") as ssp, \
             tc.tile_pool(name="po", bufs=2, space="PSUM") as pop:

            # ---- inputs, ordered by first use: xq0/wq/wk feed the QK(0)
            # lead-in, wv/xv0 the round-0 V fills, then the later chunks.
            # sync and gpsimd queues issue in parallel.
            xq_s = wgt.tile([128, NT4, 4, 2, 512], F8, tag="xq")
            xv_s = wgt.tile([128, NT4, 8, 512], F16, tag="xv")
            xq_r = xq_d.ap().rearrange(
                "p (t4 g j t) -> p t4 g j t", t4=NT4, g=4, j=2)
            xv_r = xv_d.ap().rearrange(
                "p (t4 c t) -> p t4 c t", t4=NT4, c=8)
            # PE warm-up: ~20 dummy matmuls on a zeroed scratch tile keep
            # the PE busy from ~5us (before any DMA lands) so the HAM
            # clock-gate reaches full speed before the first real matmul.
            warm = cst.tile([128, 640], F16, tag="wrm")
            nc.vector.memset(warm[:], 0.0)
            for i in range(20):
                wt = ssp.tile([128, 1024], F32, tag="ss", name=f"warm{i}")
                nc.tensor.matmul(wt[:, 0:512], warm[:, 0:128],
                                 warm[:, 128:640], start=True, stop=True)

            nc.sync.dma_start(out=xq_s[:, 0], in_=xq_r[:, 0])
            wq_s = wgt.tile([128, 4, 2, QC], F8, tag="wq")
            wk_s = wgt.tile([128, 4, 2, QC], F8, tag="wk")
            nc.scalar.dma_start(
                out=wq_s[:],
                in_=wq_d.ap().rearrange("p (g j n) -> p g j n", g=4, j=2))
            nc.scalar.dma_start(
                out=wk_s[:],
                in_=wk_d.ap().rearrange("p (g j n) -> p g j n", g=4, j=2))
            wv_s = wgt.tile([128, 8, QC], F16, tag="wv")
            nc.sync.dma_start(
                out=wv_s[:], in_=wv_d.ap().rearrange("p (c n) -> p c n", c=8))
            nc.sync.dma_start(out=xv_s[:, 0], in_=xv_r[:, 0])
            nc.sync.dma_start(out=xq_s[:, 1], in_=xq_r[:, 1])
            nc.sync.dma_start(out=xv_s[:, 1], in_=xv_r[:, 1])
            nc.scalar.dma_start(out=xq_s[:, 3], in_=xq_r[:, 3])
            nc.scalar.dma_start(out=xv_s[:, 3], in_=xv_r[:, 3])
            bq_s = cst.tile([128, 2], F32, tag="bq")
            bk_s = cst.tile([128, 2], F32, tag="bk")
            nc.gpsimd.dma_start(out=bq_s[:], in_=bq_d[:])
            nc.gpsimd.dma_start(out=bk_s[:], in_=bk_d[:])
            bv_b = cst.tile([128, QC], F32, tag="bvb")
            nc.gpsimd.dma_start(out=bv_b[:], in_=bv_d[:])
            mask = cst.tile([128, 2, 128], F16, tag="mask")
            nc.gpsimd.dma_start(
                out=mask[:], in_=mask_d.ap().rearrange("p (a n) -> p a n", a=2))
            nc.sync.dma_start(out=xq_s[:, 2], in_=xq_r[:, 2])
            nc.sync.dma_start(out=xv_s[:, 2], in_=xv_r[:, 2])
            wp_s = wgt.tile([128, 2, C], F16, tag="wp")
            nc.sync.dma_start(
                out=wp_s[:], in_=wp_d.ap().rearrange("p (j n) -> p j n", j=2))


            # ---- persistent activations ----
            qt_s = [qkp.tile([64, 2, T], F8, tag=f"qt{m}", name=f"qt{m}")
                    for m in range(2)]
            kt_s = [qkp.tile([64, 2, T], F8, tag=f"kt{m}", name=f"kt{m}")
                    for m in range(2)]
            # V': [128, kc, head, 64 v-cols + 64 ones-cols] f16
            vp_s = vvp.tile([128, NT, HPC, 2 * D], F16, tag="vp")
            nc.gpsimd.memset(vp_s[:, :, :, D:2 * D], 1.0)
            # O^T: [128, j, T] f16; head h at partitions (h%2)*64, j=h//2
            ot_s = qkp.tile([128, 2, T], F16, tag="ot")

            # ---------- emission helpers ----------
            def emit_qk(t4, m, ws, bs, dst, evac_scalar=False):
                sl = slice(t4 * 512, (t4 + 1) * 512)
                pq = mmp.tile([128, 512], F32, tag="mm")
                for g in range(4):
                    nc.tensor.matmul(
                        pq[:], ws[:, g, :, m * 128:(m + 1) * 128],
                        xq_s[:, t4, g],
                        start=(g == 0), stop=(g == 3), perf_mode=DR)
                for j in range(2):
                    if evac_scalar:
                        # lead-in only: the QK bias is per-partition, so
                        # ScalarE's activation bias port can do the PSUM
                        # evacuation while DVE handles the sibling unit --
                        # ACT is idle before the first exp
                        nc.scalar.activation(
                            dst[m][:, j, sl], pq[j * 64:(j + 1) * 64, :],
                            AF.Identity, bias=bs[j * 64:(j + 1) * 64, m:m + 1])
                    else:
                        nc.vector.tensor_scalar_add(
                            dst[m][:, j, sl], pq[j * 64:(j + 1) * 64, :],
                            bs[j * 64:(j + 1) * 64, m:m + 1])

            def emit_v(t4, i):
                # two 128-t k-chunks (kc = 4*t4+2i, +1) share one PSUM tile
                pv = mmp.tile([128, 512], F32, tag="mm")
                for ii in range(2):
                    kk = 2 * i + ii
                    for c in range(8):
                        nc.tensor.matmul(
                            pv[:, ii * QC:(ii + 1) * QC],
                            xv_s[:, t4, c, kk * 128:(kk + 1) * 128],
                            wv_s[:, c, :],
                            start=(c == 0), stop=(c == 7))
                for ii in range(2):
                    kc = 4 * t4 + 2 * i + ii
                    nc.vector.tensor_tensor(
                        vp_s[:, kc, :, 0:D],
                        pv[:, ii * QC:(ii + 1) * QC].rearrange(
                            "p (h d) -> p h d", d=D),
                        bv_b[:].rearrange("p (h d) -> p h d", d=D),
                        op=ADD)

            def emit_proj(n4, mo, dma_eng, on_scalar=False):
                lo0, hi0 = n4 * 512, (n4 + 1) * 512
                py = mmp.tile([128, 512], F32, tag="mm")
                for j in range(2):
                    nc.tensor.matmul(
                        py[:], wp_s[:, j, mo * 128:(mo + 1) * 128],
                        ot_s[:, j, lo0:hi0],
                        start=(j == 0), stop=(j == 1))
                yt_stage = yyp.tile([128, 512], F16, tag="yt")
                if on_scalar:
                    # tail only: ACT is idle there, DVE is the tail critical
                    # path
                    nc.scalar.copy(yt_stage[:], py[:])
                else:
                    nc.vector.tensor_copy(yt_stage[:], py[:])
                dma_eng.dma_start(
                    out=yt_d[mo * 128:(mo + 1) * 128, lo0:hi0], in_=yt_stage[:])

            # attention chunk emitters.  One ss PSUM tile [128,1024] per
            # k-chunk holds BOTH heads of a pair (A at cols 0:512, B at
            # 512:1024): the pair shares one exp instruction (same ACT
            # cost), and chunk-granularity buffer rotation doubles the
            # S->exp pipeline depth for the same PSUM footprint.  Diagonal
            # chunk di occupies cols 128*di:512 of each half (its natural
            # q offset), so PV writes land at the right op columns.
            def chunk_geom(t4, c):
                colA = 0 if c < 4 * t4 else 128 * (c - 4 * t4)
                return colA, 512 - colA

            def emit_S_chunk(t4, pair, c):
                """S + exp for one k-chunk, both heads of the pair; the
                two matmuls go to row groups 0/32 via tile_position and
                run concurrently in the PE array.  Attention-stream ops
                get a priority boost so ready fill work never outranks
                them in the scheduler's per-engine heaps."""
                with tc.high_priority(offset=100):
                    return _emit_S_chunk(t4, pair, c)

            def _emit_S_chunk(t4, pair, c):
                lo0, hi0 = t4 * 512, (t4 + 1) * 512
                colA, n = chunk_geom(t4, c)
                sp = ssp.tile([128, 1024], F32, tag="ss")
                pt = ppp.tile([128, 1024], F16, tag="p")
                for hh in range(2):
                    nc.tensor.matmul(
                        sp[:, 512 * hh + colA:512 * hh + colA + n],
                        kt_s[pair][hh * 32:hh * 32 + 32, :,
                                   c * 128:(c + 1) * 128],
                        qt_s[pair][hh * 32:hh * 32 + 32, :, lo0 + colA:hi0],
                        start=True, stop=True, perf_mode=DR,
                        tile_position=(hh * 32, 0))
                if colA == 0:
                    nc.scalar.activation(pt[:], sp[:], AF.Exp,
                                         scale=float(EXP_SCALE))
                else:
                    s_ap = sp[:].rearrange(
                        "p (a n) -> p a n", a=2)[:, :, colA:colA + n]
                    p_ap = pt[:].rearrange(
                        "p (a n) -> p a n", a=2)[:, :, colA:colA + n]
                    nc.scalar.activation(p_ap, s_ap, AF.Exp,
                                         scale=float(EXP_SCALE))
                if c >= 4 * t4:
                    # causal mask on the two 128-wide diagonal squares
                    # (one per head), batched into one Pool op
                    blocks = pt[:].rearrange(
                        "p (a n) -> p a n", a=2)[:, :, colA:colA + 128]
                    nc.gpsimd.tensor_tensor(blocks, blocks, mask[:], op=MUL)
                return pt

            def emit_PV_chunk(t4, h, c, pt, op_tl, is_first, is_last):
                colA, n = chunk_geom(t4, c)
                hh = h % 2
                with tc.high_priority(offset=100):
                    nc.tensor.matmul(
                    op_tl[:, colA:512], vp_s[:, c, h, :],
                    pt[:, 512 * hh + colA:512 * hh + colA + n],
                    start=is_first, stop=is_last)

            def emit_norm(t4, h, op_tl, last=False):
                with tc.high_priority(offset=100):
                    _emit_norm(t4, h, op_tl, last)

            def _emit_norm(t4, h, op_tl, last):
                m, hh = h // 2, h % 2
                sl = slice(t4 * 512, (t4 + 1) * 512)
                rc_in = dnp.tile([64, 512], F32, tag="rci")
                # the denominator staging copy runs on ScalarE: it keeps
                # the norm chain off the DVE FIFO (which fill evacuations
                # contend on) -- except for the final pair, where the ACT
                # queue is still draining exps (the copy would arrive
                # ~11us late) and DVE is the idle engine
                if last:
                    nc.vector.tensor_copy(rc_in[:], op_tl[D:2 * D, :])
                else:
                    nc.scalar.copy(rc_in[:], op_tl[D:2 * D, :])
                rc = dnp.tile([64, 512], F32, tag="rc")
                nc.vector.reciprocal_approx_fast(rc[:], rc_in[:])
                nc.vector.tensor_tensor(
                    ot_s[hh * 64:hh * 64 + 64, m, sl],
                    op_tl[0:D, :], rc[:], op=MUL)

            # ---------- main schedule ----------
            # exp on the Scalar engine is the second wall (~78us busy); in
            # the later rounds ACT paces the pipeline, so fills (QKV for a
            # later round, proj for a finished one) are pushed as late as
            # causality allows to keep the PE stream dense (HAM throttles
            # the PE clock to 1.2GHz when it sees idle gaps).
            def qk_fill(t4, m, wd, bd, dst):
                return lambda: emit_qk(t4, m, wd, bd, dst)

            def v_fill(t4, i):
                return lambda: emit_v(t4, i)

            def p_fill(t4, mo):
                return lambda: emit_proj(t4, mo, nc.sync)

            def qkv_fills(t4):
                return [qk_fill(t4, 0, wq_s, bq_s, qt_s),
                        qk_fill(t4, 0, wk_s, bk_s, kt_s),
                        qk_fill(t4, 1, wq_s, bq_s, qt_s),
                        qk_fill(t4, 1, wk_s, bk_s, kt_s),
                        v_fill(t4, 0), v_fill(t4, 1)]

            ROUND_FILLS = {
                0: qkv_fills(1),
                1: qkv_fills(2) + [p_fill(0, mo) for mo in range(2)],
                2: qkv_fills(3) + [p_fill(0, mo) for mo in range(2, 8)],
                3: [p_fill(1, mo) for mo in range(8)]
                   + [p_fill(2, mo) for mo in range(8)],
            }

            # lead-in: one dense gapless PE burst (Q/K for the pair-0
            # heads, then V(0) whose x/w land next, then the pair-1 Q/K)
            # so the HAM clock-gate flips to full speed ~3.5us in and the
            # S stream starts warm with all evacuations drained.
            emit_qk(0, 0, wq_s, bq_s, qt_s)
            emit_qk(0, 0, wk_s, bk_s, kt_s, evac_scalar=True)
            emit_v(0, 0)
            emit_v(0, 1)
            emit_qk(0, 1, wq_s, bq_s, qt_s)
            emit_qk(0, 1, wk_s, bk_s, kt_s, evac_scalar=True)

            # One flat software pipeline over every (round, pair, chunk):
            # P@V trails S/exp by 2 chunks (the ss pool depth), and the
            # trailing window crosses pair and round boundaries so the PE
            # and ACT streams never drain at a boundary.
            stream = [(t4, pair, c)
                      for t4 in range(NT4)
                      for pair in range(2)
                      for c in range(4 * t4 + 4)]
            N = len(stream)
            state = {"fills": [], "fi": 0, "slot": 0, "nslots": 1}

            def start_round(t4):
                # previous round's fills must all be emitted (QK of this
                # round is among them)
                while state["fi"] < len(state["fills"]):
                    state["fills"][state["fi"]]()
                    state["fi"] += 1
                state["fills"] = ROUND_FILLS[t4]
                state["fi"] = 0
                state["slot"] = 0
                state["nslots"] = 4 * (4 * t4 + 4) + 2

            def fill():
                # quota-spread the round's fills over its slots
                state["slot"] += 1
                quota = (len(state["fills"]) * state["slot"]
                         + state["nslots"] - 1) // state["nslots"]
                while state["fi"] < min(quota, len(state["fills"])):
                    state["fills"][state["fi"]]()
                    state["fi"] += 1

            pts = {}
            ops = {}
            for i in range(N + 2):
                if i < N:
                    t4, pair, c = stream[i]
                    if pair == 0 and c == 0:
                        start_round(t4)
                    pts[i] = emit_S_chunk(t4, pair, c)
                    fill()
                if i >= 2:
                    t4o, pairo, co = stream[i - 2]
                    ncks = 4 * t4o + 4
                    if co == 0:
                        ops[pairo] = (
                            pop.tile([128, 512], F32, tag="po",
                                     name=f"opA{t4o}_{pairo}"),
                            pop.tile([128, 512], F32, tag="po",
                                     name=f"opB{t4o}_{pairo}"))
                    opA, opB = ops[pairo]
                    pt = pts.pop(i - 2)
                    emit_PV_chunk(t4o, 2 * pairo, co, pt, opA,
                                  is_first=(co == 0), is_last=(co == ncks - 1))
                    emit_PV_chunk(t4o, 2 * pairo + 1, co, pt, opB,
                                  is_first=(co == 0), is_last=(co == ncks - 1))
                    fill()
                    if co == ncks - 1:
                        last = (t4o == NT4 - 1 and pairo == 1)
                        emit_norm(t4o, 2 * pairo, opA, last)
                        emit_norm(t4o, 2 * pairo + 1, opB, last)
            while state["fi"] < len(state["fills"]):
                state["fills"][state["fi"]]()
                state["fi"] += 1

            # tail: projection for the final round; casts alternate
            # DVE/ACT so the two engines drain the tail in parallel
            for mo in range(8):
                emit_proj(NT4 - 1, mo, nc.sync, on_scalar=(mo % 2 == 1))

    nc.finalize()
    return nc


def _get_program():
    if "nc" not in _cache:
        _ensure_env()
        _cache["nc"] = _build()
    return _cache["nc"]


def _qkv_perm():
    """Column permutation within a core's 256 q (or k) columns.

    New position m*128 + j*64 + hh*32 + dd holds original column
    (2m+hh)*64 + j*32 + dd  (m = head-pair, hh = head in pair,
    j = d//32, dd = d%32).
    """
    perm = np.empty(QC, dtype=np.int64)
    pos = 0
    for m in range(2):
        for j in range(2):
            for hh in range(2):
                for dd in range(32):
                    perm[pos] = (2 * m + hh) * 64 + j * 32 + dd
                    pos += 1
    return perm


def kernel(x, w_attn, b_attn, w_proj, b_proj):
    import ml_dtypes
    F8 = ml_dtypes.float8_e4m3

    x = np.ascontiguousarray(np.asarray(x, dtype=np.float32))
    w_attn = np.ascontiguousarray(np.asarray(w_attn, dtype=np.float32))
    b_attn = np.ascontiguousarray(np.asarray(b_attn, dtype=np.float32))
    w_proj = np.ascontiguousarray(np.asarray(w_proj, dtype=np.float32))
    b_proj = np.ascontiguousarray(np.asarray(b_proj, dtype=np.float32))

    nc = _get_program()
    from concourse.bass_utils import run_bass_kernel_spmd

    tri = np.triu(np.ones((128, 128), dtype=np.float32)).astype(np.float16)
    mask2 = np.ascontiguousarray(
        np.concatenate([tri, tri], axis=1))  # [128, 2*128]
    perm = _qkv_perm()

    xq_all, xv_all = [], []
    for b in range(B):
        xt = x[b].T.reshape(8, 128, T).transpose(1, 0, 2)  # [128, c, T]
        # chunk-major: [128, t4, c, 512]
        xt4 = xt.reshape(128, 8, 4, 512).transpose(0, 2, 1, 3)
        xq_all.append(np.ascontiguousarray(
            xt4.astype(F8).reshape(128, 8 * T)))
        xv_all.append(np.ascontiguousarray(
            xt4.astype(np.float16).reshape(128, 8 * T)))

    in_maps = []
    for c in range(NCORES):
        b = c // 4
        hg = c % 4
        q0 = hg * QC

        def wqk8(off):
            w = (WS * w_attn[:, off + q0:off + q0 + QC])[:, perm]
            w8 = w.astype(F8).reshape(4, 2, 128, QC).transpose(2, 0, 1, 3)
            return np.ascontiguousarray(w8.reshape(128, 8 * QC))

        def bqk(off):
            bb = (WS * b_attn[off + q0:off + q0 + QC])[perm]
            return np.ascontiguousarray(bb.reshape(2, 128).T.astype(np.float32))

        wv = w_attn[:, 2 * C + q0:2 * C + q0 + QC].astype(np.float16)
        wv = wv.reshape(8, 128, QC).transpose(1, 0, 2)
        wp = w_proj[q0:q0 + QC, :].astype(np.float16)
        wp = wp.reshape(2, 128, C).transpose(1, 0, 2)

        in_maps.append({
            "xq": xq_all[b],
            "xv": xv_all[b],
            "wq": wqk8(0),
            "wk": wqk8(C),
            "wv": np.ascontiguousarray(wv.reshape(128, 8 * QC)),
            "bq": bqk(0),
            "bk": bqk(C),
            "bv": np.ascontiguousarray(np.broadcast_to(
                b_attn[2 * C + q0:2 * C + q0 + QC].reshape(1, QC),
                (128, QC)).astype(np.float32)),
            "wp": np.ascontiguousarray(wp.reshape(128, 2 * C)),
            "mask": mask2,
        })

    trace = bool(os.environ.get("KERNEL_TRACE"))
    res = run_bass_kernel_spmd(nc, in_maps, list(range(NCORES)), trace=trace)
    _cache["last_results"] = res

    out = np.empty((B, T, C), dtype=np.float32)
    for b in range(B):
        acc = res.results[4 * b]["yt"].astype(np.float32)
        for c in range(4 * b + 1, 4 * b + 4):
            acc = acc + res.results[c]["yt"].astype(np.float32)
        out[b] = acc.T + b_proj
    return out



# revision 45
# speedup vs baseline: 1.0458x; 1.0458x over previous
"""Causal self-attention (B=2, T=2048, C=1024, H=16, D=64) on 8 trn2 cores.

Sharding: tensor-parallel over (batch, head-group). Core c handles batch
c//4 and heads 4*(c%4) .. 4*(c%4)+4: its 4 heads' QKV projection, causal
attention, and the partial output projection (W_proj row-shard). The 4
partials per batch are summed on the host (the Megatron all-reduce at
gather time), which also adds b_proj.

v15 design notes (~156-162us measured; v3 baseline was 204us):
  - The PE clock demotes to half speed (HAM gate) on any idle gap and
    needs ~3.5us of gapless matmuls to recover: ~20 dummy warm-up
    matmuls run during the initial DMA wait, and attention-stream ops
    carry tc.high_priority so fills never outrank them.  NOTE: the
    scheduler is chaotically sensitive -- most perturbations of this
    schedule (priorities, pool sizes, DMA engines) measured 165-197us.
  - One ss PSUM tile [128,1024] per k-chunk holds BOTH heads of a pair
    (A at 0:512, B at 512:1024): one exp instruction serves both heads
    and the chunk-granularity rotation gives S->exp depth 2 per head in
    the same 4-bank budget.  P@V trails S/exp by 2 chunks in ONE flat
    pipeline whose trailing window crosses pair and round boundaries.
  - The pair's S matmuls use EXPLICIT tile_position (hh*32, 0) (no
    auto-derive!) so they run concurrently in row groups 0/32.
  - Norm denominator staging copies run on ScalarE (off the contended
    DVE FIFO) except the final pair's, which would queue behind the
    draining exps and stall the last projection.
  - Score path in fp8 (e4m3) with DoubleRow: x^T and wq/wk are
    pre-quantized on the host (weights scaled by 32 to sit in e4m3
    normal range; the 32*32 folds into the exp scale 1/8192). Q^T/K^T
    live as [64, 2, T] f8 tiles (heads 2m,2m+1 at partition bases 0/32,
    j=d//32) so S^T runs fp8 DoubleRow with 2x32 contraction.
  - Value path in f16 end-to-end (x^T f16, wv f16, V f16, P=exp in f16,
    P@V f16, proj f16): fp8 there costs ~2-3% output error (peaked
    softmax does not average it away), over the 2e-2 gate.
  - V' carries 64 duplicated ones-columns so P@V' accumulates the
    softmax denominator in PSUM rows 64:127; normalize is
    copy/recip/multiply on DVE (reciprocal_approx_fast misreads PSUM,
    so the denominator is copied to SBUF first).
  - Diagonal-block causal masks multiply exp output on the Pool engine
    (2 blocks batched per op); DVE carries only the PSUM-touching work.
  - y^T partials leave as f32 straight from PSUM via DMA (no engine
    downcast); host sums 4 partials per batch and adds b_proj.
"""
import os
import sys
import numpy as np

B, T, C = 2, 2048, 1024
H, D = 16, 64
HPC = 4                 # heads per core
QC = HPC * D            # 256 qkv cols per core
NCORES = 8
NT = T // 128           # 16 k-chunks of 128
NT4 = T // 512          # 4 q-chunks of 512
SCALE = 1.0 / np.sqrt(D)
WS = 32.0               # fp8 pre-scale on wq/wk + biases
EXP_SCALE = SCALE / (WS * WS)   # = 1/8192

_cache = {}


def _ensure_env():
    for p in ("/opt/trn_rl_repo", "/root/.axon_site/_ro/trn_rl_repo"):
        if os.path.isdir(p) and p not in sys.path:
            sys.path.append(p)
    jp = os.environ.get("JAX_PLATFORMS")
    if jp and "axon" not in jp and "jax" not in sys.modules:
        os.environ["JAX_PLATFORMS"] = ""


def _build():
    import concourse.bass as bass
    import concourse.bacc as bacc
    import concourse.mybir as mybir
    import concourse.tile as tile

    F32 = mybir.dt.float32
    F16 = mybir.dt.float16
    F8 = mybir.dt.float8e4
    AF = mybir.ActivationFunctionType
    DR = mybir.MatmulPerfMode.DoubleRow
    MUL = bass.mybir.AluOpType.mult
    ADD = bass.mybir.AluOpType.add

    nc = bacc.Bacc()
    xq_d = nc.dram_tensor("xq", [128, 8 * T], F8, kind="ExternalInput")
    xv_d = nc.dram_tensor("xv", [128, 8 * T], F16, kind="ExternalInput")
    wq_d = nc.dram_tensor("wq", [128, 8 * QC], F8, kind="ExternalInput")
    wk_d = nc.dram_tensor("wk", [128, 8 * QC], F8, kind="ExternalInput")
    wv_d = nc.dram_tensor("wv", [128, 8 * QC], F16, kind="ExternalInput")
    bq_d = nc.dram_tensor("bq", [128, 2], F32, kind="ExternalInput")
    bk_d = nc.dram_tensor("bk", [128, 2], F32, kind="ExternalInput")
    bv_d = nc.dram_tensor("bv", [128, QC], F32, kind="ExternalInput")
    wp_d = nc.dram_tensor("wp", [128, 2 * C], F16, kind="ExternalInput")
    mask_d = nc.dram_tensor("mask", [128, 2 * 128], F16, kind="ExternalInput")
    yt_d = nc.dram_tensor("yt", [C, T], F16, kind="ExternalOutput")

    with tile.TileContext(nc) as tc:
        with tc.tile_pool(name="cst", bufs=1) as cst, \
             tc.tile_pool(name="wgt", bufs=1) as wgt, \
             tc.tile_pool(name="qk", bufs=1) as qkp, \
             tc.tile_pool(name="vv", bufs=1) as vvp, \
             tc.tile_pool(name="pp", bufs=8) as ppp, \
             tc.tile_pool(name="dn", bufs=3) as dnp, \
             tc.tile_pool(name="yy", bufs=4) as yyp, \
             tc.tile_pool(name="mm", bufs=2, space="PSUM") as mmp, \
             tc.tile_pool(name="ss", bufs=2, space="PSUM") as ssp, \
             tc.tile_pool(name="po", bufs=2, space="PSUM") as pop:

            # ---- inputs, ordered by first use: xq0/wq/wk feed the QK(0)
            # lead-in, wv/xv0 the round-0 V fills, then the later chunks.
            # sync and gpsimd queues issue in parallel.
            xq_s = wgt.tile([128, NT4, 4, 2, 512], F8, tag="xq")
            xv_s = wgt.tile([128, NT4, 8, 512], F16, tag="xv")
            xq_r = xq_d.ap().rearrange(
                "p (t4 g j t) -> p t4 g j t", t4=NT4, g=4, j=2)
            xv_r = xv_d.ap().rearrange(
                "p (t4 c t) -> p t4 c t", t4=NT4, c=8)
            # PE warm-up: ~20 dummy matmuls on a zeroed scratch tile keep
            # the PE busy from ~5us (before any DMA lands) so the HAM
            # clock-gate reaches full speed before the first real matmul.
            warm = cst.tile([128, 640], F16, tag="wrm")
            nc.vector.memset(warm[:], 0.0)
            for i in range(20):
                wt = ssp.tile([128, 1024], F32, tag="ss", name=f"warm{i}")
                nc.tensor.matmul(wt[:, 0:512], warm[:, 0:128],
                                 warm[:, 128:640], start=True, stop=True)

            nc.sync.dma_start(out=xq_s[:, 0], in_=xq_r[:, 0])
            wq_s = wgt.tile([128, 4, 2, QC], F8, tag="wq")
            wk_s = wgt.tile([128, 4, 2, QC], F8, tag="wk")
            nc.scalar.dma_start(
                out=wq_s[:],
                in_=wq_d.ap().rearrange("p (g j n) -> p g j n", g=4, j=2))
            nc.scalar.dma_start(
                out=wk_s[:],
                in_=wk_d.ap().rearrange("p (g j n) -> p g j n", g=4, j=2))
            wv_s = wgt.tile([128, 8, QC], F16, tag="wv")
            nc.sync.dma_start(
                out=wv_s[:], in_=wv_d.ap().rearrange("p (c n) -> p c n", c=8))
            nc.sync.dma_start(out=xv_s[:, 0], in_=xv_r[:, 0])
            nc.sync.dma_start(out=xq_s[:, 1], in_=xq_r[:, 1])
            nc.sync.dma_start(out=xv_s[:, 1], in_=xv_r[:, 1])
            nc.scalar.dma_start(out=xq_s[:, 3], in_=xq_r[:, 3])
            nc.scalar.dma_start(out=xv_s[:, 3], in_=xv_r[:, 3])
            bq_s = cst.tile([128, 2], F32, tag="bq")
            bk_s = cst.tile([128, 2], F32, tag="bk")
            nc.gpsimd.dma_start(out=bq_s[:], in_=bq_d[:])
            nc.gpsimd.dma_start(out=bk_s[:], in_=bk_d[:])
            bv_b = cst.tile([128, QC], F32, tag="bvb")
            nc.gpsimd.dma_start(out=bv_b[:], in_=bv_d[:])
            mask = cst.tile([128, 2, 128], F16, tag="mask")
            nc.gpsimd.dma_start(
                out=mask[:], in_=mask_d.ap().rearrange("p (a n) -> p a n", a=2))
            nc.sync.dma_start(out=xq_s[:, 2], in_=xq_r[:, 2])
            nc.sync.dma_start(out=xv_s[:, 2], in_=xv_r[:, 2])
            wp_s = wgt.tile([128, 2, C], F16, tag="wp")
            nc.sync.dma_start(
                out=wp_s[:], in_=wp_d.ap().rearrange("p (j n) -> p j n", j=2))


            # ---- persistent activations ----
            qt_s = [qkp.tile([64, 2, T], F8, tag=f"qt{m}", name=f"qt{m}")
                    for m in range(2)]
            kt_s = [qkp.tile([64, 2, T], F8, tag=f"kt{m}", name=f"kt{m}")
                    for m in range(2)]
            # V': [128, kc, head, 64 v-cols + 64 ones-cols] f16
            vp_s = vvp.tile([128, NT, HPC, 2 * D], F16, tag="vp")
            nc.gpsimd.memset(vp_s[:, :, :, D:2 * D], 1.0)
            # O^T: [128, j, T] f16; head h at partitions (h%2)*64, j=h//2
            ot_s = qkp.tile([128, 2, T], F16, tag="ot")

            # ---------- emission helpers ----------
            def emit_qk(t4, m, ws, bs, dst):
                sl = slice(t4 * 512, (t4 + 1) * 512)
                pq = mmp.tile([128, 512], F32, tag="mm")
                for g in range(4):
                    nc.tensor.matmul(
                        pq[:], ws[:, g, :, m * 128:(m + 1) * 128],
                        xq_s[:, t4, g],
                        start=(g == 0), stop=(g == 3), perf_mode=DR)
                for j in range(2):
                    nc.vector.tensor_scalar_add(
                        dst[m][:, j, sl], pq[j * 64:(j + 1) * 64, :],
                        bs[j * 64:(j + 1) * 64, m:m + 1])

            def emit_v(t4, i):
                # two 128-t k-chunks (kc = 4*t4+2i, +1) share one PSUM tile
                pv = mmp.tile([128, 512], F32, tag="mm")
                for ii in range(2):
                    kk = 2 * i + ii
                    for c in range(8):
                        nc.tensor.matmul(
                            pv[:, ii * QC:(ii + 1) * QC],
                            xv_s[:, t4, c, kk * 128:(kk + 1) * 128],
                            wv_s[:, c, :],
                            start=(c == 0), stop=(c == 7))
                for ii in range(2):
                    kc = 4 * t4 + 2 * i + ii
                    nc.vector.tensor_tensor(
                        vp_s[:, kc, :, 0:D],
                        pv[:, ii * QC:(ii + 1) * QC].rearrange(
                            "p (h d) -> p h d", d=D),
                        bv_b[:].rearrange("p (h d) -> p h d", d=D),
                        op=ADD)

            def emit_proj(n4, mo, dma_eng, on_scalar=False):
                lo0, hi0 = n4 * 512, (n4 + 1) * 512
                py = mmp.tile([128, 512], F32, tag="mm")
                for j in range(2):
                    nc.tensor.matmul(
                        py[:], wp_s[:, j, mo * 128:(mo + 1) * 128],
                        ot_s[:, j, lo0:hi0],
                        start=(j == 0), stop=(j == 1))
                yt_stage = yyp.tile([128, 512], F16, tag="yt")
                if on_scalar:
                    # tail only: ACT is idle there, DVE is the tail critical
                    # path
                    nc.scalar.copy(yt_stage[:], py[:])
                else:
                    nc.vector.tensor_copy(yt_stage[:], py[:])
                dma_eng.dma_start(
                    out=yt_d[mo * 128:(mo + 1) * 128, lo0:hi0], in_=yt_stage[:])

            # attention chunk emitters.  One ss PSUM tile [128,1024] per
            # k-chunk holds BOTH heads of a pair (A at cols 0:512, B at
            # 512:1024): the pair shares one exp instruction (same ACT
            # cost), and chunk-granularity buffer rotation doubles the
            # S->exp pipeline depth for the same PSUM footprint.  Diagonal
            # chunk di occupies cols 128*di:512 of each half (its natural
            # q offset), so PV writes land at the right op columns.
            def chunk_geom(t4, c):
                colA = 0 if c < 4 * t4 else 128 * (c - 4 * t4)
                return colA, 512 - colA

            def emit_S_chunk(t4, pair, c):
                """S + exp for one k-chunk, both heads of the pair; the
                two matmuls go to row groups 0/32 via tile_position and
                run concurrently in the PE array.  Attention-stream ops
                get a priority boost so ready fill work never outranks
                them in the scheduler's per-engine heaps."""
                with tc.high_priority(offset=100):
                    return _emit_S_chunk(t4, pair, c)

            def _emit_S_chunk(t4, pair, c):
                lo0, hi0 = t4 * 512, (t4 + 1) * 512
                colA, n = chunk_geom(t4, c)
                sp = ssp.tile([128, 1024], F32, tag="ss")
                pt = ppp.tile([128, 1024], F16, tag="p")
                for hh in range(2):
                    nc.tensor.matmul(
                        sp[:, 512 * hh + colA:512 * hh + colA + n],
                        kt_s[pair][hh * 32:hh * 32 + 32, :,
                                   c * 128:(c + 1) * 128],
                        qt_s[pair][hh * 32:hh * 32 + 32, :, lo0 + colA:hi0],
                        start=True, stop=True, perf_mode=DR,
                        tile_position=(hh * 32, 0))
                if colA == 0:
                    nc.scalar.activation(pt[:], sp[:], AF.Exp,
                                         scale=float(EXP_SCALE))
                else:
                    s_ap = sp[:].rearrange(
                        "p (a n) -> p a n", a=2)[:, :, colA:colA + n]
                    p_ap = pt[:].rearrange(
                        "p (a n) -> p a n", a=2)[:, :, colA:colA + n]
                    nc.scalar.activation(p_ap, s_ap, AF.Exp,
                                         scale=float(EXP_SCALE))
                if c >= 4 * t4:
                    # causal mask on the two 128-wide diagonal squares
                    # (one per head), batched into one Pool op
                    blocks = pt[:].rearrange(
                        "p (a n) -> p a n", a=2)[:, :, colA:colA + 128]
                    nc.gpsimd.tensor_tensor(blocks, blocks, mask[:], op=MUL)
                return pt

            def emit_PV_chunk(t4, h, c, pt, op_tl, is_first, is_last):
                colA, n = chunk_geom(t4, c)
                hh = h % 2
                with tc.high_priority(offset=100):
                    nc.tensor.matmul(
                    op_tl[:, colA:512], vp_s[:, c, h, :],
                    pt[:, 512 * hh + colA:512 * hh + colA + n],
                    start=is_first, stop=is_last)

            def emit_norm(t4, h, op_tl, last=False):
                with tc.high_priority(offset=100):
                    _emit_norm(t4, h, op_tl, last)

            def _emit_norm(t4, h, op_tl, last):
                m, hh = h // 2, h % 2
                sl = slice(t4 * 512, (t4 + 1) * 512)
                rc_in = dnp.tile([64, 512], F32, tag="rci")
                # the denominator staging copy runs on ScalarE: it keeps
                # the norm chain off the DVE FIFO (which fill evacuations
                # contend on) -- except for the final pair, where the ACT
                # queue is still draining exps (the copy would arrive
                # ~11us late) and DVE is the idle engine
                if last:
                    nc.vector.tensor_copy(rc_in[:], op_tl[D:2 * D, :])
                else:
                    nc.scalar.copy(rc_in[:], op_tl[D:2 * D, :])
                rc = dnp.tile([64, 512], F32, tag="rc")
                nc.vector.reciprocal_approx_fast(rc[:], rc_in[:])
                nc.vector.tensor_tensor(
                    ot_s[hh * 64:hh * 64 + 64, m, sl],
                    op_tl[0:D, :], rc[:], op=MUL)

            # ---------- main schedule ----------
            # exp on the Scalar engine is the second wall (~78us busy); in
            # the later rounds ACT paces the pipeline, so fills (QKV for a
            # later round, proj for a finished one) are pushed as late as
            # causality allows to keep the PE stream dense (HAM throttles
            # the PE clock to 1.2GHz when it sees idle gaps).
            def qk_fill(t4, m, wd, bd, dst):
                return lambda: emit_qk(t4, m, wd, bd, dst)

            def v_fill(t4, i):
                return lambda: emit_v(t4, i)

            def p_fill(t4, mo):
                return lambda: emit_proj(t4, mo, nc.sync)

            def qkv_fills(t4):
                return [qk_fill(t4, 0, wq_s, bq_s, qt_s),
                        qk_fill(t4, 0, wk_s, bk_s, kt_s),
                        qk_fill(t4, 1, wq_s, bq_s, qt_s),
                        qk_fill(t4, 1, wk_s, bk_s, kt_s),
                        v_fill(t4, 0), v_fill(t4, 1)]

            ROUND_FILLS = {
                0: qkv_fills(1),
                1: qkv_fills(2) + [p_fill(0, mo) for mo in range(2)],
                2: qkv_fills(3) + [p_fill(0, mo) for mo in range(2, 8)],
                3: [p_fill(1, mo) for mo in range(8)]
                   + [p_fill(2, mo) for mo in range(8)],
            }

            # lead-in: one dense gapless PE burst (Q/K for the pair-0
            # heads, then V(0) whose x/w land next, then the pair-1 Q/K)
            # so the HAM clock-gate flips to full speed ~3.5us in and the
            # S stream starts warm with all evacuations drained.
            emit_qk(0, 0, wq_s, bq_s, qt_s)
            emit_qk(0, 0, wk_s, bk_s, kt_s)
            emit_v(0, 0)
            emit_v(0, 1)
            emit_qk(0, 1, wq_s, bq_s, qt_s)
            emit_qk(0, 1, wk_s, bk_s, kt_s)

            # One flat software pipeline over every (round, pair, chunk):
            # P@V trails S/exp by 2 chunks (the ss pool depth), and the
            # trailing window crosses pair and round boundaries so the PE
            # and ACT streams never drain at a boundary.
            stream = [(t4, pair, c)
                      for t4 in range(NT4)
                      for pair in range(2)
                      for c in range(4 * t4 + 4)]
            N = len(stream)
            state = {"fills": [], "fi": 0, "slot": 0, "nslots": 1}

            def start_round(t4):
                # previous round's fills must all be emitted (QK of this
                # round is among them)
                while state["fi"] < len(state["fills"]):
                    state["fills"][state["fi"]]()
                    state["fi"] += 1
                state["fills"] = ROUND_FILLS[t4]
                state["fi"] = 0
                state["slot"] = 0
                state["nslots"] = 4 * (4 * t4 + 4) + 2

            def fill():
                # quota-spread the round's fills over its slots
                state["slot"] += 1
                quota = (len(state["fills"]) * state["slot"]
                         + state["nslots"] - 1) // state["nslots"]
                while state["fi"] < min(quota, len(state["fills"])):
                    state["fills"][state["fi"]]()
                    state["fi"] += 1

            pts = {}
            ops = {}
            for i in range(N + 2):
                if i < N:
                    t4, pair, c = stream[i]
                    if pair == 0 and c == 0:
                        start_round(t4)
                    pts[i] = emit_S_chunk(t4, pair, c)
                    fill()
                if i >= 2:
                    t4o, pairo, co = stream[i - 2]
                    ncks = 4 * t4o + 4
                    if co == 0:
                        ops[pairo] = (
                            pop.tile([128, 512], F32, tag="po",
                                     name=f"opA{t4o}_{pairo}"),
                            pop.tile([128, 512], F32, tag="po",
                                     name=f"opB{t4o}_{pairo}"))
                    opA, opB = ops[pairo]
                    pt = pts.pop(i - 2)
                    emit_PV_chunk(t4o, 2 * pairo, co, pt, opA,
                                  is_first=(co == 0), is_last=(co == ncks - 1))
                    emit_PV_chunk(t4o, 2 * pairo + 1, co, pt, opB,
                                  is_first=(co == 0), is_last=(co == ncks - 1))
                    fill()
                    if co == ncks - 1:
                        last = (t4o == NT4 - 1 and pairo == 1)
                        emit_norm(t4o, 2 * pairo, opA, last)
                        emit_norm(t4o, 2 * pairo + 1, opB, last)
            while state["fi"] < len(state["fills"]):
                state["fills"][state["fi"]]()
                state["fi"] += 1

            # tail: projection for the final round; casts alternate
            # DVE/ACT so the two engines drain the tail in parallel
            for mo in range(8):
                emit_proj(NT4 - 1, mo, nc.sync, on_scalar=(mo % 2 == 1))

    nc.finalize()
    return nc


def _get_program():
    if "nc" not in _cache:
        _ensure_env()
        _cache["nc"] = _build()
    return _cache["nc"]


def _qkv_perm():
    """Column permutation within a core's 256 q (or k) columns.

    New position m*128 + j*64 + hh*32 + dd holds original column
    (2m+hh)*64 + j*32 + dd  (m = head-pair, hh = head in pair,
    j = d//32, dd = d%32).
    """
    perm = np.empty(QC, dtype=np.int64)
    pos = 0
    for m in range(2):
        for j in range(2):
            for hh in range(2):
                for dd in range(32):
                    perm[pos] = (2 * m + hh) * 64 + j * 32 + dd
                    pos += 1
    return perm


def kernel(x, w_attn, b_attn, w_proj, b_proj):
    import ml_dtypes
    F8 = ml_dtypes.float8_e4m3

    x = np.ascontiguousarray(np.asarray(x, dtype=np.float32))
    w_attn = np.ascontiguousarray(np.asarray(w_attn, dtype=np.float32))
    b_attn = np.ascontiguousarray(np.asarray(b_attn, dtype=np.float32))
    w_proj = np.ascontiguousarray(np.asarray(w_proj, dtype=np.float32))
    b_proj = np.ascontiguousarray(np.asarray(b_proj, dtype=np.float32))

    nc = _get_program()
    from concourse.bass_utils import run_bass_kernel_spmd

    tri = np.triu(np.ones((128, 128), dtype=np.float32)).astype(np.float16)
    mask2 = np.ascontiguousarray(
        np.concatenate([tri, tri], axis=1))  # [128, 2*128]
    perm = _qkv_perm()

    xq_all, xv_all = [], []
    for b in range(B):
        xt = x[b].T.reshape(8, 128, T).transpose(1, 0, 2)  # [128, c, T]
        # chunk-major: [128, t4, c, 512]
        xt4 = xt.reshape(128, 8, 4, 512).transpose(0, 2, 1, 3)
        xq_all.append(np.ascontiguousarray(
            xt4.astype(F8).reshape(128, 8 * T)))
        xv_all.append(np.ascontiguousarray(
            xt4.astype(np.float16).reshape(128, 8 * T)))

    in_maps = []
    for c in range(NCORES):
        b = c // 4
        hg = c % 4
        q0 = hg * QC

        def wqk8(off):
            w = (WS * w_attn[:, off + q0:off + q0 + QC])[:, perm]
            w8 = w.astype(F8).reshape(4, 2, 128, QC).transpose(2, 0, 1, 3)
            return np.ascontiguousarray(w8.reshape(128, 8 * QC))

        def bqk(off):
            bb = (WS * b_attn[off + q0:off + q0 + QC])[perm]
            return np.ascontiguousarray(bb.reshape(2, 128).T.astype(np.float32))

        wv = w_attn[:, 2 * C + q0:2 * C + q0 + QC].astype(np.float16)
        wv = wv.reshape(8, 128, QC).transpose(1, 0, 2)
        wp = w_proj[q0:q0 + QC, :].astype(np.float16)
        wp = wp.reshape(2, 128, C).transpose(1, 0, 2)

        in_maps.append({
            "xq": xq_all[b],
            "xv": xv_all[b],
            "wq": wqk8(0),
            "wk": wqk8(C),
            "wv": np.ascontiguousarray(wv.reshape(128, 8 * QC)),
            "bq": bqk(0),
            "bk": bqk(C),
            "bv": np.ascontiguousarray(np.broadcast_to(
                b_attn[2 * C + q0:2 * C + q0 + QC].reshape(1, QC),
                (128, QC)).astype(np.float32)),
            "wp": np.ascontiguousarray(wp.reshape(128, 2 * C)),
            "mask": mask2,
        })

    trace = bool(os.environ.get("KERNEL_TRACE"))
    res = run_bass_kernel_spmd(nc, in_maps, list(range(NCORES)), trace=trace)
    _cache["last_results"] = res

    out = np.empty((B, T, C), dtype=np.float32)
    for b in range(B):
        acc = res.results[4 * b]["yt"].astype(np.float32)
        for c in range(4 * b + 1, 4 * b + 4):
            acc = acc + res.results[c]["yt"].astype(np.float32)
        out[b] = acc.T + b_proj
    return out



# revision 48
# speedup vs baseline: 1.0584x; 1.0121x over previous
"""Causal self-attention (B=2, T=2048, C=1024, H=16, D=64) on 8 trn2 cores.

Sharding: tensor-parallel over (batch, head-group). Core c handles batch
c//4 and heads 4*(c%4) .. 4*(c%4)+4: its 4 heads' QKV projection, causal
attention, and the partial output projection (W_proj row-shard). The 4
partials per batch are summed on the host (the Megatron all-reduce at
gather time), which also adds b_proj.

v15 design notes (~156-162us measured; v3 baseline was 204us):
  - The PE clock demotes to half speed (HAM gate) on any idle gap and
    needs ~3.5us of gapless matmuls to recover: ~20 dummy warm-up
    matmuls run during the initial DMA wait, and attention-stream ops
    carry tc.high_priority so fills never outrank them.  NOTE: the
    scheduler is chaotically sensitive -- most perturbations of this
    schedule (priorities, pool sizes, DMA engines) measured 165-197us.
  - One ss PSUM tile [128,1024] per k-chunk holds BOTH heads of a pair
    (A at 0:512, B at 512:1024): one exp instruction serves both heads
    and the chunk-granularity rotation gives S->exp depth 2 per head in
    the same 4-bank budget.  P@V trails S/exp by 2 chunks in ONE flat
    pipeline whose trailing window crosses pair and round boundaries.
  - The pair's S matmuls use EXPLICIT tile_position (hh*32, 0) (no
    auto-derive!) so they run concurrently in row groups 0/32.
  - Norm denominator staging copies run on ScalarE (off the contended
    DVE FIFO) except the final pair's, which would queue behind the
    draining exps and stall the last projection.
  - Score path in fp8 (e4m3) with DoubleRow: x^T and wq/wk are
    pre-quantized on the host (weights scaled by 32 to sit in e4m3
    normal range; the 32*32 folds into the exp scale 1/8192). Q^T/K^T
    live as [64, 2, T] f8 tiles (heads 2m,2m+1 at partition bases 0/32,
    j=d//32) so S^T runs fp8 DoubleRow with 2x32 contraction.
  - Value path in f16 end-to-end (x^T f16, wv f16, V f16, P=exp in f16,
    P@V f16, proj f16): fp8 there costs ~2-3% output error (peaked
    softmax does not average it away), over the 2e-2 gate.
  - V' carries 64 duplicated ones-columns so P@V' accumulates the
    softmax denominator in PSUM rows 64:127; normalize is
    copy/recip/multiply on DVE (reciprocal_approx_fast misreads PSUM,
    so the denominator is copied to SBUF first).
  - Diagonal-block causal masks multiply exp output on the Pool engine
    (2 blocks batched per op); DVE carries only the PSUM-touching work.
  - y^T partials leave as f32 straight from PSUM via DMA (no engine
    downcast); host sums 4 partials per batch and adds b_proj.
"""
import os
import sys
import numpy as np

B, T, C = 2, 2048, 1024
H, D = 16, 64
HPC = 4                 # heads per core
QC = HPC * D            # 256 qkv cols per core
NCORES = 8
NT = T // 128           # 16 k-chunks of 128
NT4 = T // 512          # 4 q-chunks of 512
SCALE = 1.0 / np.sqrt(D)
WS = 32.0               # fp8 pre-scale on wq/wk + biases
EXP_SCALE = SCALE / (WS * WS)   # = 1/8192

_cache = {}


def _ensure_env():
    for p in ("/opt/trn_rl_repo", "/root/.axon_site/_ro/trn_rl_repo"):
        if os.path.isdir(p) and p not in sys.path:
            sys.path.append(p)
    jp = os.environ.get("JAX_PLATFORMS")
    if jp and "axon" not in jp and "jax" not in sys.modules:
        os.environ["JAX_PLATFORMS"] = ""


def _build():
    import concourse.bass as bass
    import concourse.bacc as bacc
    import concourse.mybir as mybir
    import concourse.tile as tile

    F32 = mybir.dt.float32
    F16 = mybir.dt.float16
    F8 = mybir.dt.float8e4
    AF = mybir.ActivationFunctionType
    DR = mybir.MatmulPerfMode.DoubleRow
    MUL = bass.mybir.AluOpType.mult
    ADD = bass.mybir.AluOpType.add

    nc = bacc.Bacc()
    xq_d = nc.dram_tensor("xq", [128, 8 * T], F8, kind="ExternalInput")
    xv_d = nc.dram_tensor("xv", [128, 8 * T], F16, kind="ExternalInput")
    wq_d = nc.dram_tensor("wq", [128, 8 * QC], F8, kind="ExternalInput")
    wk_d = nc.dram_tensor("wk", [128, 8 * QC], F8, kind="ExternalInput")
    wv_d = nc.dram_tensor("wv", [128, 8 * QC], F16, kind="ExternalInput")
    bq_d = nc.dram_tensor("bq", [128, 2], F32, kind="ExternalInput")
    bk_d = nc.dram_tensor("bk", [128, 2], F32, kind="ExternalInput")
    bv_d = nc.dram_tensor("bv", [128, QC], F32, kind="ExternalInput")
    wp_d = nc.dram_tensor("wp", [128, 2 * C], F16, kind="ExternalInput")
    mask_d = nc.dram_tensor("mask", [128, 2 * 128], F16, kind="ExternalInput")
    yt_d = nc.dram_tensor("yt", [C, T], F16, kind="ExternalOutput")

    with tile.TileContext(nc) as tc:
        with tc.tile_pool(name="cst", bufs=1) as cst, \
             tc.tile_pool(name="wgt", bufs=1) as wgt, \
             tc.tile_pool(name="qk", bufs=1) as qkp, \
             tc.tile_pool(name="vv", bufs=1) as vvp, \
             tc.tile_pool(name="pp", bufs=8) as ppp, \
             tc.tile_pool(name="dn", bufs=3) as dnp, \
             tc.tile_pool(name="yy", bufs=4) as yyp, \
             tc.tile_pool(name="mm", bufs=2, space="PSUM") as mmp, \
             tc.tile_pool(name="ss", bufs=2, space="PSUM") as ssp, \
             tc.tile_pool(name="po", bufs=2, space="PSUM") as pop:

            # ---- inputs, ordered by first use: xq0/wq/wk feed the QK(0)
            # lead-in, wv/xv0 the round-0 V fills, then the later chunks.
            # sync and gpsimd queues issue in parallel.
            xq_s = wgt.tile([128, NT4, 4, 2, 512], F8, tag="xq")
            xv_s = wgt.tile([128, NT4, 8, 512], F16, tag="xv")
            xq_r = xq_d.ap().rearrange(
                "p (t4 g j t) -> p t4 g j t", t4=NT4, g=4, j=2)
            xv_r = xv_d.ap().rearrange(
                "p (t4 c t) -> p t4 c t", t4=NT4, c=8)
            # PE warm-up: ~20 dummy matmuls on a zeroed scratch tile keep
            # the PE busy from ~5us (before any DMA lands) so the HAM
            # clock-gate reaches full speed before the first real matmul.
            warm = cst.tile([128, 640], F16, tag="wrm")
            nc.vector.memset(warm[:], 0.0)
            for i in range(20):
                wt = ssp.tile([128, 1024], F32, tag="ss", name=f"warm{i}")
                nc.tensor.matmul(wt[:, 0:512], warm[:, 0:128],
                                 warm[:, 128:640], start=True, stop=True)

            nc.sync.dma_start(out=xq_s[:, 0], in_=xq_r[:, 0])
            wq_s = wgt.tile([128, 4, 2, QC], F8, tag="wq")
            wk_s = wgt.tile([128, 4, 2, QC], F8, tag="wk")
            nc.scalar.dma_start(
                out=wq_s[:],
                in_=wq_d.ap().rearrange("p (g j n) -> p g j n", g=4, j=2))
            nc.scalar.dma_start(
                out=wk_s[:],
                in_=wk_d.ap().rearrange("p (g j n) -> p g j n", g=4, j=2))
            wv_s = wgt.tile([128, 8, QC], F16, tag="wv")
            nc.sync.dma_start(
                out=wv_s[:], in_=wv_d.ap().rearrange("p (c n) -> p c n", c=8))
            nc.sync.dma_start(out=xv_s[:, 0], in_=xv_r[:, 0])
            nc.sync.dma_start(out=xq_s[:, 1], in_=xq_r[:, 1])
            nc.sync.dma_start(out=xv_s[:, 1], in_=xv_r[:, 1])
            nc.scalar.dma_start(out=xq_s[:, 3], in_=xq_r[:, 3])
            nc.scalar.dma_start(out=xv_s[:, 3], in_=xv_r[:, 3])
            bq_s = cst.tile([128, 2], F32, tag="bq")
            bk_s = cst.tile([128, 2], F32, tag="bk")
            nc.gpsimd.dma_start(out=bq_s[:], in_=bq_d[:])
            nc.gpsimd.dma_start(out=bk_s[:], in_=bk_d[:])
            bv_b = cst.tile([128, QC], F32, tag="bvb")
            nc.gpsimd.dma_start(out=bv_b[:], in_=bv_d[:])
            mask = cst.tile([128, 2, 128], F16, tag="mask")
            nc.gpsimd.dma_start(
                out=mask[:], in_=mask_d.ap().rearrange("p (a n) -> p a n", a=2))
            nc.sync.dma_start(out=xq_s[:, 2], in_=xq_r[:, 2])
            nc.sync.dma_start(out=xv_s[:, 2], in_=xv_r[:, 2])
            wp_s = wgt.tile([128, 2, C], F16, tag="wp")
            nc.sync.dma_start(
                out=wp_s[:], in_=wp_d.ap().rearrange("p (j n) -> p j n", j=2))


            # ---- persistent activations ----
            qt_s = [qkp.tile([64, 2, T], F8, tag=f"qt{m}", name=f"qt{m}")
                    for m in range(2)]
            kt_s = [qkp.tile([64, 2, T], F8, tag=f"kt{m}", name=f"kt{m}")
                    for m in range(2)]
            # V': [128, kc, head, 64 v-cols + 64 ones-cols] f16
            vp_s = vvp.tile([128, NT, HPC, 2 * D], F16, tag="vp")
            nc.gpsimd.memset(vp_s[:, :, :, D:2 * D], 1.0)
            # O^T: [128, j, T] f16; head h at partitions (h%2)*64, j=h//2
            ot_s = qkp.tile([128, 2, T], F16, tag="ot")

            # ---------- emission helpers ----------
            def emit_qk(t4, m, ws, bs, dst):
                sl = slice(t4 * 512, (t4 + 1) * 512)
                pq = mmp.tile([128, 512], F32, tag="mm")
                for g in range(4):
                    nc.tensor.matmul(
                        pq[:], ws[:, g, :, m * 128:(m + 1) * 128],
                        xq_s[:, t4, g],
                        start=(g == 0), stop=(g == 3), perf_mode=DR)
                for j in range(2):
                    nc.vector.tensor_scalar_add(
                        dst[m][:, j, sl], pq[j * 64:(j + 1) * 64, :],
                        bs[j * 64:(j + 1) * 64, m:m + 1])

            def emit_v(t4, i):
                # two 128-t k-chunks (kc = 4*t4+2i, +1) share one PSUM tile
                pv = mmp.tile([128, 512], F32, tag="mm")
                for ii in range(2):
                    kk = 2 * i + ii
                    for c in range(8):
                        nc.tensor.matmul(
                            pv[:, ii * QC:(ii + 1) * QC],
                            xv_s[:, t4, c, kk * 128:(kk + 1) * 128],
                            wv_s[:, c, :],
                            start=(c == 0), stop=(c == 7))
                for ii in range(2):
                    kc = 4 * t4 + 2 * i + ii
                    nc.vector.tensor_tensor(
                        vp_s[:, kc, :, 0:D],
                        pv[:, ii * QC:(ii + 1) * QC].rearrange(
                            "p (h d) -> p h d", d=D),
                        bv_b[:].rearrange("p (h d) -> p h d", d=D),
                        op=ADD)

            def emit_proj(n4, mo, dma_eng, on_scalar=False):
                lo0, hi0 = n4 * 512, (n4 + 1) * 512
                py = mmp.tile([128, 512], F32, tag="mm")
                for j in range(2):
                    nc.tensor.matmul(
                        py[:], wp_s[:, j, mo * 128:(mo + 1) * 128],
                        ot_s[:, j, lo0:hi0],
                        start=(j == 0), stop=(j == 1))
                yt_stage = yyp.tile([128, 512], F16, tag="yt")
                if on_scalar:
                    # tail only: ACT is idle there, DVE is the tail critical
                    # path
                    nc.scalar.copy(yt_stage[:], py[:])
                else:
                    nc.vector.tensor_copy(yt_stage[:], py[:])
                dma_eng.dma_start(
                    out=yt_d[mo * 128:(mo + 1) * 128, lo0:hi0], in_=yt_stage[:])

            # attention chunk emitters.  One ss PSUM tile [128,1024] per
            # k-chunk holds BOTH heads of a pair (A at cols 0:512, B at
            # 512:1024): the pair shares one exp instruction (same ACT
            # cost), and chunk-granularity buffer rotation doubles the
            # S->exp pipeline depth for the same PSUM footprint.  Diagonal
            # chunk di occupies cols 128*di:512 of each half (its natural
            # q offset), so PV writes land at the right op columns.
            def chunk_geom(t4, c):
                colA = 0 if c < 4 * t4 else 128 * (c - 4 * t4)
                return colA, 512 - colA

            def emit_S_chunk(t4, pair, c):
                """S + exp for one k-chunk, both heads of the pair; the
                two matmuls go to row groups 0/32 via tile_position and
                run concurrently in the PE array.  Attention-stream ops
                get a priority boost so ready fill work never outranks
                them in the scheduler's per-engine heaps."""
                with tc.high_priority(offset=100):
                    return _emit_S_chunk(t4, pair, c)

            def _emit_S_chunk(t4, pair, c):
                lo0, hi0 = t4 * 512, (t4 + 1) * 512
                colA, n = chunk_geom(t4, c)
                sp = ssp.tile([128, 1024], F32, tag="ss")
                pt = ppp.tile([128, 1024], F16, tag="p")
                for hh in range(2):
                    nc.tensor.matmul(
                        sp[:, 512 * hh + colA:512 * hh + colA + n],
                        kt_s[pair][hh * 32:hh * 32 + 32, :,
                                   c * 128:(c + 1) * 128],
                        qt_s[pair][hh * 32:hh * 32 + 32, :, lo0 + colA:hi0],
                        start=True, stop=True, perf_mode=DR,
                        tile_position=(hh * 32, 0))
                if colA == 0:
                    nc.scalar.activation(pt[:], sp[:], AF.Exp,
                                         scale=float(EXP_SCALE))
                else:
                    s_ap = sp[:].rearrange(
                        "p (a n) -> p a n", a=2)[:, :, colA:colA + n]
                    p_ap = pt[:].rearrange(
                        "p (a n) -> p a n", a=2)[:, :, colA:colA + n]
                    nc.scalar.activation(p_ap, s_ap, AF.Exp,
                                         scale=float(EXP_SCALE))
                if c >= 4 * t4:
                    # causal mask on the two 128-wide diagonal squares
                    # (one per head), batched into one Pool op
                    blocks = pt[:].rearrange(
                        "p (a n) -> p a n", a=2)[:, :, colA:colA + 128]
                    nc.gpsimd.tensor_tensor(blocks, blocks, mask[:], op=MUL)
                return pt

            def emit_PV_chunk(t4, h, c, pt, op_tl, is_first, is_last):
                colA, n = chunk_geom(t4, c)
                hh = h % 2
                with tc.high_priority(offset=100):
                    nc.tensor.matmul(
                    op_tl[:, colA:512], vp_s[:, c, h, :],
                    pt[:, 512 * hh + colA:512 * hh + colA + n],
                    start=is_first, stop=is_last)

            def emit_norm(t4, h, op_tl, last=False):
                with tc.high_priority(offset=100):
                    _emit_norm(t4, h, op_tl, last)

            def _emit_norm(t4, h, op_tl, last):
                m, hh = h // 2, h % 2
                sl = slice(t4 * 512, (t4 + 1) * 512)
                rc_in = dnp.tile([64, 512], F32, tag="rci")
                # the denominator staging copy runs on ScalarE: it keeps
                # the norm chain off the DVE FIFO (which fill evacuations
                # contend on) -- except for the final pair, where the ACT
                # queue is still draining exps (the copy would arrive
                # ~11us late) and DVE is the idle engine
                if last:
                    nc.vector.tensor_copy(rc_in[:], op_tl[D:2 * D, :])
                else:
                    nc.scalar.copy(rc_in[:], op_tl[D:2 * D, :])
                rc = dnp.tile([64, 512], F32, tag="rc")
                nc.vector.reciprocal_approx_fast(rc[:], rc_in[:])
                nc.vector.tensor_tensor(
                    ot_s[hh * 64:hh * 64 + 64, m, sl],
                    op_tl[0:D, :], rc[:], op=MUL)

            # ---------- main schedule ----------
            # exp on the Scalar engine is the second wall (~78us busy); in
            # the later rounds ACT paces the pipeline, so fills (QKV for a
            # later round, proj for a finished one) are pushed as late as
            # causality allows to keep the PE stream dense (HAM throttles
            # the PE clock to 1.2GHz when it sees idle gaps).
            def qk_fill(t4, m, wd, bd, dst):
                return lambda: emit_qk(t4, m, wd, bd, dst)

            def v_fill(t4, i):
                return lambda: emit_v(t4, i)

            def p_fill(t4, mo):
                return lambda: emit_proj(t4, mo, nc.sync)

            def qkv_fills(t4):
                return [qk_fill(t4, 0, wq_s, bq_s, qt_s),
                        qk_fill(t4, 0, wk_s, bk_s, kt_s),
                        qk_fill(t4, 1, wq_s, bq_s, qt_s),
                        qk_fill(t4, 1, wk_s, bk_s, kt_s),
                        v_fill(t4, 0), v_fill(t4, 1)]

            ROUND_FILLS = {
                0: qkv_fills(1),
                1: qkv_fills(2) + [p_fill(0, mo) for mo in range(2)],
                2: qkv_fills(3) + [p_fill(0, mo) for mo in range(2, 8)],
                3: [p_fill(1, mo) for mo in range(8)]
                   + [p_fill(2, mo) for mo in range(8)],
            }

            # lead-in: one dense gapless PE burst (Q/K for the pair-0
            # heads, then V(0) whose x/w land next, then the pair-1 Q/K)
            # so the HAM clock-gate flips to full speed ~3.5us in and the
            # S stream starts warm with all evacuations drained.
            emit_qk(0, 0, wq_s, bq_s, qt_s)
            emit_qk(0, 0, wk_s, bk_s, kt_s)
            emit_v(0, 0)
            emit_v(0, 1)
            emit_qk(0, 1, wq_s, bq_s, qt_s)
            emit_qk(0, 1, wk_s, bk_s, kt_s)

            # One flat software pipeline over every (round, pair, chunk):
            # P@V trails S/exp by 2 chunks (the ss pool depth), and the
            # trailing window crosses pair and round boundaries so the PE
            # and ACT streams never drain at a boundary.
            stream = [(t4, pair, c)
                      for t4 in range(NT4)
                      for pair in range(2)
                      for c in range(4 * t4 + 4)]
            N = len(stream)
            state = {"fills": [], "fi": 0, "slot": 0, "nslots": 1}

            def start_round(t4):
                # previous round's fills must all be emitted (QK of this
                # round is among them)
                while state["fi"] < len(state["fills"]):
                    state["fills"][state["fi"]]()
                    state["fi"] += 1
                state["fills"] = ROUND_FILLS[t4]
                state["fi"] = 0
                state["slot"] = 0
                state["nslots"] = 4 * (4 * t4 + 4) + 2

            def fill():
                # quota-spread the round's fills over its slots
                state["slot"] += 1
                quota = (len(state["fills"]) * state["slot"]
                         + state["nslots"] - 1) // state["nslots"]
                while state["fi"] < min(quota, len(state["fills"])):
                    state["fills"][state["fi"]]()
                    state["fi"] += 1

            pts = {}
            ops = {}
            for i in range(N + 2):
                if i < N:
                    t4, pair, c = stream[i]
                    if pair == 0 and c == 0:
                        start_round(t4)
                    pts[i] = emit_S_chunk(t4, pair, c)
                    fill()
                if i >= 2:
                    t4o, pairo, co = stream[i - 2]
                    ncks = 4 * t4o + 4
                    if co == 0:
                        ops[pairo] = (
                            pop.tile([128, 512], F32, tag="po",
                                     name=f"opA{t4o}_{pairo}"),
                            pop.tile([128, 512], F32, tag="po",
                                     name=f"opB{t4o}_{pairo}"))
                    opA, opB = ops[pairo]
                    pt = pts.pop(i - 2)
                    emit_PV_chunk(t4o, 2 * pairo, co, pt, opA,
                                  is_first=(co == 0), is_last=(co == ncks - 1))
                    emit_PV_chunk(t4o, 2 * pairo + 1, co, pt, opB,
                                  is_first=(co == 0), is_last=(co == ncks - 1))
                    fill()
                    if co == ncks - 1:
                        last = (t4o == NT4 - 1 and pairo == 1)
                        emit_norm(t4o, 2 * pairo, opA, last)
                        emit_norm(t4o, 2 * pairo + 1, opB, last)
            while state["fi"] < len(state["fills"]):
                state["fills"][state["fi"]]()
                state["fi"] += 1

            # tail: projection for the final round; casts alternate
            # DVE/ACT so the two engines drain the tail in parallel
            for mo in range(8):
                emit_proj(NT4 - 1, mo, nc.sync, on_scalar=(mo % 2 == 1))

    nc.finalize()
    return nc


def _get_program():
    if "nc" not in _cache:
        _ensure_env()
        _cache["nc"] = _build()
    return _cache["nc"]


def _qkv_perm():
    """Column permutation within a core's 256 q (or k) columns.

    New position m*128 + j*64 + hh*32 + dd holds original column
    (2m+hh)*64 + j*32 + dd  (m = head-pair, hh = head in pair,
    j = d//32, dd = d%32).
    """
    perm = np.empty(QC, dtype=np.int64)
    pos = 0
    for m in range(2):
        for j in range(2):
            for hh in range(2):
                for dd in range(32):
                    perm[pos] = (2 * m + hh) * 64 + j * 32 + dd
                    pos += 1
    return perm


def kernel(x, w_attn, b_attn, w_proj, b_proj):
    import ml_dtypes
    F8 = ml_dtypes.float8_e4m3

    x = np.ascontiguousarray(np.asarray(x, dtype=np.float32))
    w_attn = np.ascontiguousarray(np.asarray(w_attn, dtype=np.float32))
    b_attn = np.ascontiguousarray(np.asarray(b_attn, dtype=np.float32))
    w_proj = np.ascontiguousarray(np.asarray(w_proj, dtype=np.float32))
    b_proj = np.ascontiguousarray(np.asarray(b_proj, dtype=np.float32))

    nc = _get_program()
    from concourse.bass_utils import run_bass_kernel_spmd

    tri = np.triu(np.ones((128, 128), dtype=np.float32)).astype(np.float16)
    mask2 = np.ascontiguousarray(
        np.concatenate([tri, tri], axis=1))  # [128, 2*128]
    perm = _qkv_perm()

    xq_all, xv_all = [], []
    for b in range(B):
        xt = x[b].T.reshape(8, 128, T).transpose(1, 0, 2)  # [128, c, T]
        # chunk-major: [128, t4, c, 512]
        xt4 = xt.reshape(128, 8, 4, 512).transpose(0, 2, 1, 3)
        xq_all.append(np.ascontiguousarray(
            xt4.astype(F8).reshape(128, 8 * T)))
        xv_all.append(np.ascontiguousarray(
            xt4.astype(np.float16).reshape(128, 8 * T)))

    in_maps = []
    for c in range(NCORES):
        b = c // 4
        hg = c % 4
        q0 = hg * QC

        def wqk8(off):
            w = (WS * w_attn[:, off + q0:off + q0 + QC])[:, perm]
            w8 = w.astype(F8).reshape(4, 2, 128, QC).transpose(2, 0, 1, 3)
            return np.ascontiguousarray(w8.reshape(128, 8 * QC))

        def bqk(off):
            bb = (WS * b_attn[off + q0:off + q0 + QC])[perm]
            return np.ascontiguousarray(bb.reshape(2, 128).T.astype(np.float32))

        wv = w_attn[:, 2 * C + q0:2 * C + q0 + QC].astype(np.float16)
        wv = wv.reshape(8, 128, QC).transpose(1, 0, 2)
        wp = w_proj[q0:q0 + QC, :].astype(np.float16)
        wp = wp.reshape(2, 128, C).transpose(1, 0, 2)

        in_maps.append({
            "xq": xq_all[b],
            "xv": xv_all[b],
            "wq": wqk8(0),
            "wk": wqk8(C),
            "wv": np.ascontiguousarray(wv.reshape(128, 8 * QC)),
            "bq": bqk(0),
            "bk": bqk(C),
            "bv": np.ascontiguousarray(np.broadcast_to(
                b_attn[2 * C + q0:2 * C + q0 + QC].reshape(1, QC),
                (128, QC)).astype(np.float32)),
            "wp": np.ascontiguousarray(wp.reshape(128, 2 * C)),
            "mask": mask2,
        })

    trace = bool(os.environ.get("KERNEL_TRACE"))
    res = run_bass_kernel_spmd(nc, in_maps, list(range(NCORES)), trace=trace)
    _cache["last_results"] = res

    out = np.empty((B, T, C), dtype=np.float32)
    for b in range(B):
        acc = res.results[4 * b]["yt"].astype(np.float32)
        for c in range(4 * b + 1, 4 * b + 4):
            acc = acc + res.results[c]["yt"].astype(np.float32)
        out[b] = acc.T + b_proj
    return out

